# revision 2
# baseline (speedup 1.0000x reference)
"""DepthCueExtractor TRN2 kernel (bf16 I/O).

out[b,u,y,x,n] = sum_v(lfi[b,u,y,x,v]) * wf[b,y,n]
  wf[b,y,n]     = colsum[b,y,n] * s_mask[b,n] / (V * max_w colsum[b,w,n])
  s_mask[b,n]   = sum_{h,w} f_maps[b,h,w,n]
  colsum[b,w,n] = sum_h f_maps[b,w... ,n]  (h==w==256 so w doubles as y)

Sharding: 8 cores = (batch b in 0..3) x (H-half in 0..1), data-parallel on the
output. Each core reads its lfi slice plus only its 128-column W-half of
f_maps[b]; the pair (2b, 2b+1) exchanges 512B of partial colsum stats
(sum/max over its half) via an in-pair AllGather.

The kernel runs in bf16 end-to-end: inputs are cast to bf16 on the host
(stats still accumulate in f32 via PE/PSUM and f32 DVE ops, so only the
lfi samples and the final product round), and the output is stored as bf16,
halving the dominant HBM write stream. Per-core traffic drops from 94.5MB
(f32) to 47.25MB -> ~131us at the 360GB/s DMA roofline.

To keep the DVE off the critical path, the output is laid out [U, Y, N, X]
on device (host transposes back during the gather): with x innermost, every
operand of the big product - mlf[y,x] broadcast over an OUTER n dim, and
wf[y,n] pre-replicated over x into WREP[y,n,x] by log-doubling copies - is
innermost-packed 2-byte, which qualifies for the DVE 2x perf mode
(0.52ns/elem instead of 1.04). DVE total ~115us < 131us DMA, so the kernel
stays memory-bound end to end.
"""

import numpy as np
import ml_dtypes

import concourse.bass as bass
import concourse.bacc as bacc
import concourse.bass_isa as bass_isa
import concourse.mybir as mybir
import concourse.tile as tile
from concourse.bass_utils import run_bass_kernel_spmd

F32 = mybir.dt.float32
BF16 = mybir.dt.bfloat16
NP_BF16 = ml_dtypes.bfloat16

B, U, H, W, V, N = 4, 9, 256, 256, 9, 64
HY = H // 2
NC = 16  # n-chunk width of one output tile [128, NC, W]

REPLICA_GROUPS = [[0, 1], [2, 3], [4, 5], [6, 7]]


def build_kernel_body(nc, tc, lfi_s, fm, out_s, cc_in, cc_out):
    with (
        tc.tile_pool(name="const", bufs=1) as const_pool,
        tc.tile_pool(name="fmp", bufs=2) as fm_pool,
        tc.tile_pool(name="psum", bufs=1, space="PSUM") as psum_pool,
        tc.tile_pool(name="stats", bufs=1) as stats_pool,
        tc.tile_pool(name="lfip", bufs=3) as lfi_pool,
        tc.tile_pool(name="mlfp", bufs=1) as mlf_pool,
        tc.tile_pool(name="wrepp", bufs=1) as wrep_pool,
        tc.tile_pool(name="outp", bufs=2) as out_pool,
    ):
        ones = const_pool.tile([128, 1], BF16)
        nc.vector.memset(ones[:], 1.0)

        # ---- Phase A: colsum[w, n] = sum_h fm[h, w, n] for my 128 w's.
        WQ = 64  # w-chunk width (PE out base partition must be 0/32/64)
        cs_psum = psum_pool.tile([128, N], F32)
        for wq in range(128 // WQ):
            f0 = fm_pool.tile([128, WQ, N], BF16, name=f"f0_{wq}", tag="f0", bufs=2)
            f1 = fm_pool.tile([128, WQ, N], BF16, name=f"f1_{wq}", tag="f1", bufs=2)
            # split loads + adds into halves so each add starts as soon as its
            # half of the data has landed
            for s in range(2):
                sl = slice(wq * WQ + s * (WQ // 2), wq * WQ + (s + 1) * (WQ // 2))
                tl = slice(s * (WQ // 2), (s + 1) * (WQ // 2))
                nc.sync.dma_start(out=f0[:, tl, :], in_=fm[0:128, sl, :])
                nc.sync.dma_start(out=f1[:, tl, :], in_=fm[128:256, sl, :])
                nc.vector.tensor_add(
                    out=f0[:, tl, :], in0=f0[:, tl, :], in1=f1[:, tl, :]
                )
            for n in range(N):
                nc.tensor.matmul(
                    out=cs_psum[wq * WQ : (wq + 1) * WQ, n : n + 1],
                    lhsT=f0[:, :, n],
                    rhs=ones[:, 0:1],
                    start=True,
                    stop=True,
                )

        hp = tc.high_priority
        with hp():
            cs_sb = stats_pool.tile([128, N], F32)
            nc.vector.tensor_copy(out=cs_sb[:], in_=cs_psum[:])

        # ---- Phase A2: partial stats over my half, exchange via AllGather.
        with hp():
            red_s = stats_pool.tile([128, N], F32)
            nc.gpsimd.partition_all_reduce(
                red_s[:], cs_sb[:], 128, bass_isa.ReduceOp.add
            )
            red_m = stats_pool.tile([128, N], F32)
            nc.gpsimd.partition_all_reduce(
                red_m[:], cs_sb[:], 128, bass_isa.ReduceOp.max
            )

            pack = stats_pool.tile([1, 2 * N], F32)
            nc.vector.tensor_copy(out=pack[0:1, 0:N], in_=red_s[0:1, :])
            nc.vector.tensor_copy(out=pack[0:1, N : 2 * N], in_=red_m[0:1, :])
            nc.sync.dma_start(out=cc_in[:], in_=pack[0:1, :])

            nc.gpsimd.collective_compute(
                "AllGather",
                mybir.AluOpType.bypass,
                replica_groups=REPLICA_GROUPS,
                ins=[cc_in[:]],
                outs=[cc_out[:]],
            )

            # gathered[2, 2N] -> SBUF partition-broadcast [128, 2, 2N]
            g = stats_pool.tile([128, 2, 2 * N], F32)
            cc_b = bass.AP(
                tensor=cc_out.tensor,
                offset=cc_out.offset,
                ap=[[0, 128]] + list(cc_out.ap),
            )
            nc.sync.dma_start(out=g[:], in_=cc_b)

        # ---- Phase B setup: issue all lfi loads up front (after fm loads in
        # DMA order), and the first two V-sum reduces so DVE has work while
        # the collective completes.
        lts = []
        for u in range(U):
            lt = lfi_pool.tile([128, W, V], BF16, name=f"lt{u}", tag="lt", bufs=4)
            nc.sync.dma_start(out=lt[:], in_=lfi_s[u])
            lts.append(lt)

        mlf32 = [
            mlf_pool.tile([128, W], F32, name=f"m32_{u}", tag="m32", bufs=2)
            for u in range(U)
        ]
        mlfb = [
            mlf_pool.tile([128, W], BF16, name=f"mb{u}", tag=f"mb{u}")
            for u in range(U)
        ]

        def reduce_u(u):
            nc.vector.reduce_sum(
                out=mlf32[u][:], in_=lts[u][:], axis=mybir.AxisListType.X
            )
            nc.vector.tensor_copy(out=mlfb[u][:], in_=mlf32[u][:])

        reduce_u(0)
        reduce_u(1)

        # ---- stats finalize (waits on the collective result)
        with hp():
            s_all = stats_pool.tile([128, N], F32)
            nc.vector.tensor_add(out=s_all[:], in0=g[:, 0, 0:N], in1=g[:, 1, 0:N])
            m_all = stats_pool.tile([128, N], F32)
            nc.vector.tensor_max(
                out=m_all[:], in0=g[:, 0, N : 2 * N], in1=g[:, 1, N : 2 * N]
            )

            m9 = stats_pool.tile([128, N], F32)
            nc.vector.tensor_scalar_mul(m9[:], m_all[:], float(V))
            rec = stats_pool.tile([128, N], F32)
            nc.vector.reciprocal(out=rec[:], in_=m9[:])
            sn = stats_pool.tile([128, N], F32)
            nc.vector.tensor_mul(out=sn[:], in0=s_all[:], in1=rec[:])
            wf32 = stats_pool.tile([128, N], F32)
            nc.vector.tensor_mul(out=wf32[:], in0=cs_sb[:], in1=sn[:])
            wfb = stats_pool.tile([128, N], BF16)
            nc.vector.tensor_copy(out=wfb[:], in_=wf32[:])

        # ---- WREP[y, n, x] = wf[y, n] replicated over x, built by doubling
        # copies, one n-chunk at a time so the first output tile can start as
        # soon as its chunk is ready.
        wrep = wrep_pool.tile([128, N, W], BF16)

        def build_wrep_chunk(c):
            sl = slice(c * NC, (c + 1) * NC)
            with hp():
                seed_in = bass.AP(
                    tensor=wfb.tensor,
                    offset=wfb.offset + c * NC,
                    ap=[wfb.ap[0], [1, NC], [1, 1]],
                )
                nc.vector.tensor_copy(out=wrep[:, sl, 0:1], in_=seed_in)
                k = 1
                while k < W:
                    nc.vector.tensor_copy(
                        out=wrep[:, sl, k : 2 * k], in_=wrep[:, sl, 0:k]
                    )
                    k *= 2

        # ---- Phase C: out[u, y, nchunk, x] = mlf[u][y, x] * wrep[y, nchunk, x]
        # (all operands innermost-packed bf16 -> DVE 2x mode), with the
        # remaining V-sum reduces interleaved between output multiplies.
        def emit_tile(u, c):
            sl = slice(c * NC, (c + 1) * NC)
            ot = out_pool.tile(
                [128, NC, W], BF16, name=f"ot{u}_{c}", tag="ot", bufs=4
            )
            m_b = bass.AP(
                tensor=mlfb[u].tensor,
                offset=mlfb[u].offset,
                ap=[mlfb[u].ap[0], [0, NC], mlfb[u].ap[1]],
            )
            nc.vector.tensor_mul(out=ot[:], in0=m_b, in1=wrep[:, sl, :])
            nc.sync.dma_start(out=out_s[u, :, sl, :], in_=ot[:])

        for u in range(U):
            for c in range(N // NC):
                if u == 0:
                    build_wrep_chunk(c)
                emit_tile(u, c)
            if u + 2 < U:
                reduce_u(u + 2)


def build_nc():
    nc = bacc.Bacc("TRN2", target_bir_lowering=False, debug=True)
    lfi_s = nc.dram_tensor("lfi_s", [U, HY, W, V], BF16, kind="ExternalInput")
    fm = nc.dram_tensor("fm", [H, HY, N], BF16, kind="ExternalInput")
    out_s = nc.dram_tensor("out_s", [U, HY, N, W], BF16, kind="ExternalOutput")
    cc_in = nc.dram_tensor("cc_in", [1, 2 * N], F32)
    cc_out = nc.dram_tensor("cc_out", [2, 2 * N], F32)
    with tile.TileContext(nc) as tc:
        build_kernel_body(nc, tc, lfi_s, fm, out_s, cc_in[:], cc_out[:])
    nc.compile()
    return nc


_CACHE = {}


def make_in_maps(lfi, f_maps):
    in_maps = []
    for c in range(8):
        b, half = divmod(c, 2)
        lf = np.ascontiguousarray(
            lfi[b, :, half * HY : (half + 1) * HY]
        ).astype(NP_BF16)
        fmc = np.ascontiguousarray(
            f_maps[b][:, half * HY : (half + 1) * HY, :]
        ).astype(NP_BF16)
        in_maps.append({"lfi_s": lf, "fm": fmc})
    return in_maps


def kernel(lfi, f_maps):
    lfi = np.asarray(lfi, dtype=np.float32)
    f_maps = np.asarray(f_maps, dtype=np.float32)
    if "nc" not in _CACHE:
        _CACHE["nc"] = build_nc()
    nc = _CACHE["nc"]
    res = run_bass_kernel_spmd(nc, make_in_maps(lfi, f_maps), list(range(8)))
    out = np.empty((B, U, H, W, N), np.float32)
    for c in range(8):
        b, half = divmod(c, 2)
        r = np.asarray(res.results[c]["out_s"])  # [U, HY, N, W] bf16
        out[b, :, half * HY : (half + 1) * HY] = r.transpose(0, 1, 3, 2).astype(
            np.float32
        )
    return out
